# revision 40
# baseline (speedup 1.0000x reference)
"""Trainium2 Bass kernel for nn_CausalDecayMemory — fast banded path.

Reference (B=4, T=4096, D=512):
    q = x @ Wq.T ; k = x @ Wk.T ; v = x @ Wv.T
    scores[b,t,s] = q[b,t] . k[b,s]
    weights[t,s] = decay^max(s-t-1, 0) for s > t else 0
    out = ((scores * weights) @ v) @ Wo.T * out_scale

Algebraic folding (host-side, free):
    scores = x G x^T       with G = Wq^T Wk
    out    = (A x) H       with H = Wv^T Wo^T * out_scale,  A = scores*weights
This removes two of the four dense projections.

Decay truncation: gamma = sigmoid(decay_logit).  With 128-blocks over t/s,
an s-block only contributes to t-blocks within ND super-diagonals, where
gamma^(128*ND) < tol.  For the graded regime (gamma ~ 0.9526) ND = 1; the
dominant error is the 32-col cross-tile drop (~7e-3 rel) + bf16 (~3.5e-3),
measured 8.2e-3 total vs the 2e-2 gate.

Sharding: pure data-parallel, 8 cores = 4 batches x 2 halves of 2048
positions, each with an ND*128-position lookahead halo of x.  No collectives.

Per-core h-fused pipeline (algo="h", all matmuls bf16, fp32 PSUM):
    g^T[d,t]  = sum_e G[e,d] x^T[e,t]                  (proj 1; N=512)
    h[s,o]    = sum_e x[s,e] H[e,o]  per s-block       (proj 2; N=512,
                depends only on the input DMA -> pure filler work)
    sc[s,t]   = sum_d x^T[d,s] g^T[d,t], banded        (N~224)
    at        = sc * mask                              (DVE, bf16 out)
    out[t,o]  = sum_{j in [tb,tb+ND]} at_j^T h_j       (N=512, 2 MMs/block)
The only x layout needed is x^T (one packed 2.2MB DMA/body); the classic
rt-path (algo="rt") needs x twice (x^T and x-natural, 4.4MB) and an extra
PSUM round-trip rt = at . x, out = rt @ H.

I/O uses partition-major packed DRAM blobs (one contiguous 17KB run per
partition) for full-rate single-descriptor DMA; input x is triple-buffered
in the bench loop so each body's DMA issues ~2.7 bodies ahead of first use.
"""

import os
import sys

import numpy as np

for _p in ("/opt/trn_rl_repo",):
    if _p not in sys.path and os.path.isdir(_p):
        sys.path.insert(0, _p)

import concourse.bass as bass  # noqa: E402
import concourse.mybir as mybir  # noqa: E402
import concourse.tile as tile  # noqa: E402
from concourse import bacc  # noqa: E402
from concourse.bass_utils import run_bass_kernel_spmd  # noqa: E402

B, T, D = 4, 4096, 512
P = 128
TB = 16            # local 128-blocks per core (2048 positions)
TL = TB * P        # 2048
DB = D // P        # 4
N_CORES = 8

F32 = mybir.dt.float32
BF16 = mybir.dt.bfloat16
BF_NP = mybir.dt.np(BF16)

_BUILD_CACHE: dict = {}
LAST_RESULTS = None

ND_MAX = 3         # fast path handles up to 3 super-diagonal blocks


ALL_PARTS = frozenset({"dma_in", "compute", "copies", "dma_out"})
# dev-only: "dma_decoupled" (with "dma_in") redirects input DMAs into
# dummy tiles so compute never depends on them (contention probe).

BENCH_BODIES = 6   # unrolled bodies per bench-loop iteration (== x bufs)


def _build_fast(ND: int, bench_loop: int = 1, parts: frozenset = ALL_PARTS,
                unroll: int = 1, algo: str = "h"):
    """algo="h": h-fused pipeline out[t,o] = sum_j at_j . h_j with
    h = x @ H precomputed per s-block (no xn input, no rt stage).
    algo="rt": classic rt = at . x then out = rt @ H."""
    key = ("fast", ND, bench_loop, parts, unroll, algo)
    if key in _BUILD_CACHE:
        return _BUILD_CACHE[key]

    SBK = TB + ND          # s-blocks incl halo
    TLE = SBK * P          # extended positions
    NW = (ND + 1) * P      # mask / at width in t-columns
    USE_H = algo == "h"
    assert not (USE_H and ND != 1), "h-fused schedule is ND=1 only"

    nc = bacc.Bacc("TRN2", target_bir_lowering=False, debug=False)

    # Partition-major packed DRAM layouts: contiguous multi-KB runs per
    # partition -> full-rate DMA (1KB-line rearranges halve throughput).
    # xT is packed as NCH time-staggered chunks so each chunk's refill WAR clears
    # when ITS last reader finishes (spread through the prior body on
    # the same buffer slot) — the write traffic trickles in spread
    # bursts instead of one 2.2MB wall at body start.
    NCH = 2
    TC2 = TLE // NCH
    xT = nc.dram_tensor("xT", [P, NCH, DB * TC2], BF16,
                        kind="ExternalInput").ap()
    xn = (None if USE_H else
          nc.dram_tensor("xn", [P, SBK * D], BF16, kind="ExternalInput").ap())
    Gm = nc.dram_tensor("Gm", [D, D], BF16, kind="ExternalInput").ap()
    Hm = nc.dram_tensor("Hm", [D, D], BF16, kind="ExternalInput").ap()
    msk = nc.dram_tensor("msk", [P, NW], F32, kind="ExternalInput").ap()
    out = nc.dram_tensor("out", [P, TB * D], BF16, kind="ExternalOutput").ap()

    xT_t = xT.rearrange("p k (eo t) -> p k eo t", eo=DB)  # [128, NCH, 4, TC2]
    xn_t = (None if USE_H else
            xn.rearrange("p (sb d) -> p sb d", sb=SBK))  # [128, SBK, 512]
    G_t = Gm.rearrange("(eo p) d -> p eo d", p=P)
    H_t = Hm.rearrange("(eo p) d -> p eo d", p=P)
    out_t = out.rearrange("p (tb d) -> p tb d", tb=TB)  # [128, 16, 512]

    # input x multi-buffer (NBODY-1 body DMA lead)
    NBUF = BENCH_BODIES if (bench_loop > 1 or unroll > 1) else 1

    with tile.TileContext(nc) as tc:
        with (
            tc.tile_pool(name="cpool", bufs=1) as cpool,
            tc.tile_pool(name="dpool", bufs=NBUF) as dpool,
            tc.tile_pool(name="ppa", bufs=6, space="PSUM") as ppa,
            tc.tile_pool(name="ppo", bufs=2, space="PSUM") as ppo,
        ):
            mult = mybir.AluOpType.mult

            G_sb = cpool.tile([P, DB, D], BF16)
            H_sb = cpool.tile([P, DB, D], BF16)
            msk_sb = cpool.tile([P, NW], F32)
            g_sb = cpool.tile([P, DB, TL], BF16)
            at_sb = cpool.tile([P, SBK, NW], BF16)
            if USE_H:
                h_sb = cpool.tile([P, SBK, D], BF16)
                rt_sb = None
            else:
                h_sb = None
                rt_sb = cpool.tile([P, DB, TL], BF16)
            o_sb = cpool.tile([P, TB, D], BF16)

            # t-block coverage of s-block j
            def _cov(j):
                jt0 = max(0, j - ND)
                jt1 = min(TB, j + 1)
                off = (jt0 - j + ND) * P
                return jt0, jt1, off

            DO_DIN = "dma_in" in parts
            DO_MM = "compute" in parts
            DO_CP = "copies" in parts
            DO_DOUT = "dma_out" in parts
            DMA_DECOUPLED = "dma_decoupled" in parts

            # Stripped dev variants: initialize tiles whose producers are
            # disabled (once, outside any bench loop — slope cancels it).
            fixed_x = None
            if DMA_DECOUPLED or not DO_DIN:
                for t in (G_sb, H_sb, msk_sb):
                    nc.vector.memset(t, 0)
                fixed_x = (cpool.tile([P, DB, TLE], BF16, name="fxT"),
                           (None if USE_H else
                            cpool.tile([P, SBK, D], BF16, name="fxn")))
                for t in fixed_x:
                    if t is not None:
                        nc.vector.memset(t, 0)
            if not DO_CP:
                for t in (g_sb, at_sb, rt_sb, h_sb, o_sb):
                    if t is not None:
                        nc.vector.memset(t, 0)

            # The first 32 t-columns of each super-diagonal at-tile hold
            # only weights <= gamma^96 (~8.5e-3): never computed by _sc.
            # The h-fused out-stage reads them as part of full-M lhsT
            # slices, so pin them to zero once here.
            if USE_H and DO_CP:
                nc.vector.memset(at_sb[:, :, 0:32], 0)

            # Loop-invariant constants: one DMA, outside any bench loop.
            if "dma_in" in parts:
                nc.sync.dma_start(G_sb, G_t)
                nc.sync.dma_start(msk_sb, msk)
                nc.sync.dma_start(H_sb, H_t)

            _it = [0]

            def _alloc_x():
                # One rotation slot per call (tag-keyed); with NBUF=3 and
                # three bodies per loop iteration, each slot's refill DMA
                # issues ~2.7 bodies before its consumer.
                it = _it[0]
                _it[0] += 1
                if fixed_x is not None and not DMA_DECOUPLED:
                    return fixed_x
                if DMA_DECOUPLED:
                    return (dpool.tile([P, DB, TLE], BF16, tag="dT",
                                       name=f"dT_sb_{it}"),
                            None if USE_H else
                            dpool.tile([P, SBK, D], BF16, tag="dn",
                                       name=f"dn_sb_{it}"))
                return (dpool.tile([P, DB, TLE], BF16, tag="xT",
                                   name=f"xT_sb_{it}"),
                        None if USE_H else
                        dpool.tile([P, SBK, D], BF16, tag="xn",
                                   name=f"xn_sb_{it}"))

            def _dma_x(tiles):
                # DMA triggers only on the SP and gpsimd/Pool queues — no
                # compute queue ever stalls on a DMA WAR-wait. The NCH
                # chunks (contiguous DRAM source, strided SBUF dest) are
                # spread across BOTH rings: one ring measures only ~180GB/s,
                # so parallel rings halve the per-body transfer time.
                if DO_DIN:
                    dT_sb, dn_sb = tiles
                    for k in range(NCH):
                        eng = nc.sync if k % 2 == 0 else nc.gpsimd
                        eng.dma_start(
                            dT_sb[:, :, k * TC2:(k + 1) * TC2], xT_t[:, k])
                    if dn_sb is not None:
                        nc.gpsimd.dma_start(dn_sb, xn_t)

            def _body(x_tiles):
                if fixed_x is not None:
                    xT_sb, xn_sb = fixed_x
                else:
                    xT_sb, xn_sb = x_tiles

                def _g(c):
                    cr = slice(c * 512, (c + 1) * 512)
                    for do in range(DB):
                        pg = ppa.tile([P, 512], F32, tag="pa",
                                      name=f"pg_{c}_{do}")
                        if DO_MM:
                            for e in range(DB):
                                nc.tensor.matmul(
                                    pg, G_sb[:, e, do * P:(do + 1) * P],
                                    xT_sb[:, e, cr],
                                    start=(e == 0), stop=(e == DB - 1))
                        if DO_CP:
                            nc.vector.tensor_copy(out=g_sb[:, do, cr], in_=pg)

                def _sc(j):
                    jt0, jt1, off = _cov(j)
                    n = (jt1 - jt0) * P
                    # skip the first 32 t-cols of super-diagonal tiles
                    # (weight <= gamma^96; at_sb pinned to zero there)
                    lo = off + 32 if off == 0 else off
                    ps = ppa.tile([P, 512], F32, tag="pa", name=f"ps_{j}")
                    if DO_MM:
                        for d in range(DB):
                            nc.tensor.matmul(
                                ps[:, lo:off + n],
                                xT_sb[:, d, j * P:(j + 1) * P],
                                g_sb[:, d, jt0 * P + lo - off:jt1 * P],
                                start=(d == 0), stop=(d == DB - 1))
                    if DO_CP:
                        nc.vector.tensor_tensor(
                            out=at_sb[:, j, lo:off + n],
                            in0=ps[:, lo:off + n],
                            in1=msk_sb[:, lo:off + n], op=mult)

                def _rt(blist):
                    if not blist:
                        return
                    nb = len(blist)
                    for do in range(DB):
                        pr = ppa.tile([P, 512], F32, tag="pa",
                                      name=f"pr_{blist[0]}_{do}")
                        if DO_MM:
                            for ii, i in enumerate(blist):
                                for j in range(i, i + ND + 1):
                                    # skip the zero region: the first 32
                                    # t-cols of super-diagonal at-tiles are
                                    # never computed by _sc
                                    lo = 32 if (j == i + ND and ND > 0) else 0
                                    nc.tensor.matmul(
                                        pr[:, ii * P + lo:(ii + 1) * P],
                                        xn_sb[:, j, do * P:(do + 1) * P],
                                        at_sb[:, j,
                                              (i - j + ND) * P + lo:
                                              (i - j + ND + 1) * P],
                                        start=(j == i), stop=(j == i + ND))
                        if DO_CP:
                            nc.scalar.copy(
                                out=rt_sb[:, do,
                                          blist[0] * P:blist[0] * P + nb * P],
                                in_=pr[:, :nb * P])

                def _out(blist):
                    for tb in blist:
                        po = ppo.tile([P, D], F32, tag="po", name=f"po_{tb}")
                        if DO_MM:
                            for do in range(DB):
                                nc.tensor.matmul(
                                    po, rt_sb[:, do, tb * P:(tb + 1) * P],
                                    H_sb[:, do, :],
                                    start=(do == 0), stop=(do == DB - 1))
                        if DO_CP:
                            nc.scalar.copy(out=o_sb[:, tb, :], in_=po)
                    if DO_DOUT and blist:
                        b0, b1 = blist[0], blist[-1] + 1
                        nc.gpsimd.dma_start(out_t[:, b0:b1, :],
                                            o_sb[:, b0:b1, :])

                def _h(j):
                    # h[s, o] = sum_e x[s, e] H[e, o] for s-block j
                    ph = ppa.tile([P, D], F32, tag="pa", name=f"ph_{j}")
                    if DO_MM:
                        for e in range(DB):
                            nc.tensor.matmul(
                                ph, xT_sb[:, e, j * P:(j + 1) * P],
                                H_sb[:, e, :],
                                start=(e == 0), stop=(e == DB - 1))
                    if DO_CP:
                        nc.scalar.copy(out=h_sb[:, j, :], in_=ph)

                def _out2(blist):
                    # out[t, o] = sum_{j in [tb, tb+ND]} at_j[:, tcols].T @ h_j
                    # (at's zero-pinned cols contribute 0 to the first rows)
                    for tb in blist:
                        po = ppo.tile([P, D], F32, tag="po", name=f"po_{tb}")
                        if DO_MM:
                            for j in range(tb, tb + ND + 1):
                                nc.tensor.matmul(
                                    po,
                                    at_sb[:, j, (tb - j + ND) * P:
                                          (tb - j + ND + 1) * P],
                                    h_sb[:, j, :],
                                    start=(j == tb), stop=(j == tb + ND))
                        if DO_CP:
                            nc.scalar.copy(out=o_sb[:, tb, :], in_=po)
                    if DO_DOUT and blist:
                        b0, b1 = blist[0], blist[-1] + 1
                        nc.gpsimd.dma_start(out_t[:, b0:b1, :],
                                            o_sb[:, b0:b1, :])

                if USE_H:
                    # h(j) depends only on the input DMA -> pure filler work.
                    # sc emissions are interleaved with h so the DVE (which
                    # drains one at-mult per ~660ns) is never asked for more
                    # than one per ~1.2us of PE work, and every _out2 batch
                    # has >=1.5us of PE slack after the at/h tiles it reads.
                    _g(0)
                    _g(1)
                    _h(0)
                    _sc(0)
                    _h(1)
                    _sc(1)
                    _h(2)
                    _sc(2)
                    _h(3)
                    _sc(3)
                    _h(4)
                    _g(2)
                    _sc(4)
                    _h(5)
                    _sc(5)
                    _h(6)
                    _sc(6)
                    _out2([0, 1])
                    _sc(7)
                    _out2([2, 3])
                    _g(3)
                    _sc(8)
                    _h(7)
                    _sc(9)
                    _h(8)
                    _sc(10)
                    _h(9)
                    _out2([4, 5])
                    _sc(11)
                    _h(10)
                    _out2([6, 7])
                    _sc(12)
                    _h(11)
                    _sc(13)
                    _h(12)
                    _sc(14)
                    _h(13)
                    _out2([8, 9])
                    _sc(15)
                    _h(14)
                    for j in range(TB, TB + ND):
                        _sc(j)
                    _h(15)
                    _h(16)
                    _out2([10, 11, 12, 13])
                    _out2([14, 15])
                else:
                    # rt batches are shifted by ND blocks so batch k only
                    # needs at-tiles <= 4k+3 (emitted just before).
                    rtg = [list(range(max(0, 4 * k - ND), 4 * (k + 1) - ND))
                           for k in range(4)] + [list(range(16 - ND, 16))]
                    _g(0)
                    _g(1)
                    for j in range(0, 4):
                        _sc(j)
                    for c in (1, 2, 3):
                        if c < 3:
                            _g(c + 1)
                        _rt(rtg[c - 1])
                        for j in range(4 * c, 4 * c + 4):
                            _sc(j)
                        if c == 3:
                            for j in range(TB, TB + ND):
                                _sc(j)
                        _out(rtg[c - 1])
                    _rt(rtg[3])
                    _rt(rtg[4])
                    _out(rtg[3])
                    _out(rtg[4])

            if bench_loop > 1:
                # BENCH_BODIES unrolled bodies per hardware iteration with
                # as many x-buffers: each body's input DMA is issued
                # ~NBODY-1 bodies before its first consumer, so transfers
                # overlap compute, and any conservative per-iteration loop
                # sync is amortized over NBODY bodies.
                # Effective executions per run: BENCH_BODIES * bench_loop.
                hint = (mybir.EngineType.PE, mybir.EngineType.DVE,
                        mybir.EngineType.Activation, mybir.EngineType.SP,
                        mybir.EngineType.Pool)
                ring = []
                for _ in range(BENCH_BODIES - 1):
                    t = _alloc_x()
                    _dma_x(t)
                    ring.append(t)
                with tc.For_i(0, bench_loop, 1, hint_engines=hint):
                    for i in range(BENCH_BODIES):
                        t = _alloc_x()
                        _dma_x(t)
                        ring.append(t)
                        _body(ring[i])
            else:
                for _ in range(unroll):
                    a = _alloc_x()
                    _dma_x(a)
                    _body(a)

    nc.compile()
    _BUILD_CACHE[key] = nc
    return nc


# ---------------------------------------------------------------------------
# Exact fallback path (v0): RetNet-style chunked-decay recurrence with
# carried KV state.  Used only when gamma is too close to 1 for the
# banded fast path (ND > ND_MAX).  Verbatim from the previous kernel.
# ---------------------------------------------------------------------------
C = 512          # super-chunk length
NS = 4           # 128-sub-tiles per 512
NL = 4           # local super-chunks per core (2048 positions)

# Matmul input dtype: float32r streams 4x faster than float32 on the PE at
# N>=256 (single-pass relaxed-precision fp32); same bit layout as fp32.
# KERNEL_DT: "f32r" (default) | "f32" | "bf16"
_DT_MODE = os.environ.get("KERNEL_DT",
                          "f32" if os.environ.get("KERNEL_F32") == "1"
                          else "f32r")
USE_F32R = _DT_MODE == "f32r"



MD = {"f32r": mybir.dt.float32r, "f32": F32,
      "bf16": mybir.dt.bfloat16}[_DT_MODE]  # matmul-input dtype
MD_NP = mybir.dt.np(MD)


TUNE = {
    "ppa": 4, "ppr": 4, "kt": "mm", "eng": "vec", "odma": "sync", "obufs": 1,
    "work": 2, "proj": 2, "state": 2,
}


def _build_v0(NE: int, has_state: bool, bench_loop: int = 1, tune: dict | None = None,
           cs_trim: bool = True):
    """Build + compile the per-core Bass program. NE = total super-chunks
    (NL local + lookahead tail); has_state = carry decayed KV state across
    chunks (exact for any gamma) vs. single-chunk truncation. bench_loop > 1
    wraps the body in an on-device loop (timing use only)."""
    tn = dict(TUNE)
    if tune:
        tn.update(tune)
    key = (NE, has_state, _DT_MODE, bench_loop, cs_trim, tuple(sorted(tn.items())))
    if key in _BUILD_CACHE:
        return _BUILD_CACHE[key]

    nc = bacc.Bacc("TRN2", target_bir_lowering=False, debug=False)

    xT = nc.dram_tensor("xT", [D, NE * C], MD, kind="ExternalInput").ap()
    wqT = nc.dram_tensor("wqT", [D, D], MD, kind="ExternalInput").ap()
    wkT = nc.dram_tensor("wkT", [D, D], MD, kind="ExternalInput").ap()
    wvT = nc.dram_tensor("wvT", [D, D], MD, kind="ExternalInput").ap()
    woTs = nc.dram_tensor("woTs", [D, D], MD, kind="ExternalInput").ap()
    m3 = nc.dram_tensor("m3", [C, C], F32, kind="ExternalInput").ap()
    qsc = nc.dram_tensor("qsc", [P, C], F32, kind="ExternalInput").ap()
    ksc = nc.dram_tensor("ksc", [P, NS], F32, kind="ExternalInput").ap()
    ksc2 = nc.dram_tensor("ksc2", [P, C], F32, kind="ExternalInput").ap()
    idn = nc.dram_tensor("idn", [P, P], MD, kind="ExternalInput").ap()
    idc = nc.dram_tensor("idc", [P, P], MD, kind="ExternalInput").ap()
    out = nc.dram_tensor("out", [NL * C, D], F32, kind="ExternalOutput").ap()

    xT_t = xT.rearrange("(eo p) t -> p eo t", p=P)          # [128, 4, NE*C]
    wq_t = wqT.rearrange("(eo p) d -> p eo d", p=P)
    wk_t = wkT.rearrange("(eo p) d -> p eo d", p=P)
    wv_t = wvT.rearrange("(eo p) d -> p eo d", p=P)
    wo_t = woTs.rearrange("(eo p) d -> p eo d", p=P)
    m3_t = m3.rearrange("(so p) t -> p so t", p=P)
    out_t = out.rearrange("(c ts p) d -> p c ts d", p=P, ts=NS)

    with tile.TileContext(nc) as tc:
        with (
            tc.tile_pool(name="wpool", bufs=1) as wpool,
            tc.tile_pool(name="cpool", bufs=1) as cpool,
            tc.tile_pool(name="state", bufs=tn["state"]) as state,
            tc.tile_pool(name="proj", bufs=tn["proj"]) as proj,
            tc.tile_pool(name="work", bufs=tn["work"]) as work,
            tc.tile_pool(name="ppa", bufs=tn["ppa"], space="PSUM") as ppa,
            tc.tile_pool(name="ppr", bufs=tn["ppr"], space="PSUM") as ppr,
        ):
            mult = mybir.AluOpType.mult
            _rr = [0]

            def _eng():
                if tn["eng"] == "any":
                    return nc.any
                if tn["eng"] == "vec":
                    return nc.vector
                _rr[0] ^= 1
                return nc.vector if _rr[0] else nc.scalar

            def _cp(out, in_):
                e = _eng()
                if e is nc.scalar:
                    nc.scalar.copy(out=out, in_=in_)
                else:
                    e.tensor_copy(out=out, in_=in_)

            def _tt(out, in0, in1):
                e = _eng()
                if e is nc.scalar:
                    e = nc.vector   # ACT has no general tensor_tensor
                e.tensor_tensor(out=out, in0=in0, in1=in1, op=mult)

            wq_sb = wpool.tile([P, NS, D], MD)
            nc.sync.dma_start(wq_sb, wq_t)
            wk_sb = wpool.tile([P, NS, D], MD)
            nc.sync.dma_start(wk_sb, wk_t)
            wv_sb = wpool.tile([P, NS, D], MD)
            nc.sync.dma_start(wv_sb, wv_t)
            wo_sb = wpool.tile([P, NS, D], MD)
            nc.sync.dma_start(wo_sb, wo_t)
            m3_sb = cpool.tile([P, NS, C], F32)
            nc.sync.dma_start(m3_sb, m3_t)
            qsc_sb = cpool.tile([P, C], F32)
            nc.sync.dma_start(qsc_sb, qsc)
            ksc_sb = cpool.tile([P, NS], F32)
            nc.sync.dma_start(ksc_sb, ksc)
            ksc2_sb = cpool.tile([P, C], F32)
            nc.sync.dma_start(ksc2_sb, ksc2)
            idn_sb = cpool.tile([P, P], MD)
            nc.sync.dma_start(idn_sb, idn)
            idc_sb = cpool.tile([P, P], MD)
            nc.sync.dma_start(idc_sb, idc)

            def _chunks():
                kv_prev = None   # (kT, v) [fast] or (kscaled, v) [general]
                S_prev = None    # state tile (general path only)
                # triangular trim: scores/intra block so only needs
                # t in (so*128 - 256, (so+1)*128) -- the decay window bound
                # applies below as well when cs_trim; keep N >= 256 for
                # full-rate fp32r
                if cs_trim and not has_state:
                    TRIM = [(0, 256), (0, 256), (0, 384), (P, 384)]
                else:
                    TRIM = [(0, max(256, (so + 1) * P)) for so in range(NS)]
                for c in range(NE - 1, -1, -1):
                    local = c < NL
                    need_kv = c > 0 or local

                    halo_trim = (not has_state) and cs_trim and c == NE - 1
                    nh = C // 2 if halo_trim else C
                    xt = work.tile([P, NS, C], MD, tag="xt", name=f"xt_{c}")
                    nc.sync.dma_start(xt[:, :, :nh],
                                      xT_t[:, :, c * C:c * C + nh])

                    # ---- general path: scaled-natural k + decayed state S ----
                    if has_state and kv_prev is not None:
                        ksc_p, v_p = kv_prev
                        S_cur = state.tile([P, NS, D], MD, tag="S", name=f"S_{c}")
                        for eo in range(NS):
                            ps = ppa.tile([P, D], F32, tag="pa", name=f"psS_{c}_{eo}")
                            with_id = S_prev is not None
                            for so in range(NS):
                                nc.tensor.matmul(
                                    ps,
                                    ksc_p[:, so, eo * P:(eo + 1) * P],
                                    v_p[:, so, :],
                                    start=(so == 0),
                                    stop=(so == NS - 1 and not with_id),
                                )
                            if with_id:
                                nc.tensor.matmul(
                                    ps, idc_sb, S_prev[:, eo, :],
                                    start=False, stop=True,
                                )
                            _cp(S_cur[:, eo, :], ps)
                        S_prev = S_cur

                    if has_state and need_kv:
                        ksc_c = proj.tile([P, NS, D], MD, tag="ksc", name=f"ksc_{c}")
                        for so in range(NS):
                            pk = ppa.tile([P, D], F32, tag="pa", name=f"psk_{c}_{so}")
                            for eo in range(NS):
                                nc.tensor.matmul(
                                    pk,
                                    xt[:, eo, so * P:(so + 1) * P],
                                    wk_sb[:, eo, :],
                                    start=(eo == 0), stop=(eo == NS - 1),
                                )
                            _tt(ksc_c[:, so, :], pk,
                                ksc_sb[:, so:so + 1].to_broadcast((P, D)))

                    # ---- shared: v natural; scaled k^T (fast: all chunks) ----
                    if need_kv:
                        n_vso = (NS // 2 if ((not has_state) and cs_trim
                                             and c == NE - 1) else NS)
                        v_c = proj.tile([P, NS, D], MD, tag="v", name=f"v_{c}")
                        for so in range(n_vso):
                            pv = ppa.tile([P, D], F32, tag="pa", name=f"psv_{c}_{so}")
                            for eo in range(NS):
                                nc.tensor.matmul(
                                    pv,
                                    xt[:, eo, so * P:(so + 1) * P],
                                    wv_sb[:, eo, :],
                                    start=(eo == 0), stop=(eo == NS - 1),
                                )
                            _cp(v_c[:, so, :], pv)

                    # halo chunk only feeds the cross path, whose weight
                    # is < gamma^256 beyond its first 256 positions
                    if need_kv and (local or not has_state):
                        kt_c = work.tile([P, NS, C], MD, tag="kt", name=f"kt_{c}")
                        for do in range(NS):
                            pk2 = ppa.tile([P, C], F32, tag="pa",
                                           name=f"pskt_{c}_{do}")
                            for ei in range(NS):
                                nc.tensor.matmul(
                                    pk2[:, :nh],
                                    wk_sb[:, ei, do * P:(do + 1) * P],
                                    xt[:, ei, :nh],
                                    start=(ei == 0), stop=(ei == NS - 1),
                                )
                            _tt(kt_c[:, do, :nh], pk2[:, :nh], ksc2_sb[:, :nh])

                    if local:
                        # scaled q^T: qt[e, t] with gamma^(C-1-i) folded in
                        qt_c = work.tile([P, NS, C], MD, tag="qt", name=f"qt_{c}")
                        for eo in range(NS):
                            pq = ppa.tile([P, C], F32, tag="pa", name=f"psq_{c}_{eo}")
                            for ei in range(NS):
                                nc.tensor.matmul(
                                    pq,
                                    wq_sb[:, ei, eo * P:(eo + 1) * P],
                                    xt[:, ei, :],
                                    start=(ei == 0), stop=(ei == NS - 1),
                                )
                            _tt(qt_c[:, eo, :], pq, qsc_sb)

                        # fast path: cross-chunk scores cs[s', t] =
                        # (K~_prev Q~_c) using the transposed k of chunk c+1;
                        # cross then becomes V_prev^T @ cs (no natural k, no S)
                        if not has_state:
                            # cross weight <= gamma^(C - TC) for t < TC, so
                            # the t < TC half can be dropped when gamma is
                            # small enough (cs_trim)
                            TC = C // 2 if cs_trim else 0
                            NC_ = C - TC
                            kt_p, v_p = kv_prev
                            n_prev = (NS // 2 if (cs_trim and c == NL - 1
                                                  and NE == NL + 1) else NS)
                            cs_sb = state.tile([P, NS, C], MD, tag="S",
                                               name=f"cs_{c}")
                            for so in range(n_prev):
                                pcs = ppa.tile([P, C], F32, tag="pa",
                                               name=f"pscs_{c}_{so}")
                                for dk in range(NS):
                                    nc.tensor.matmul(
                                        pcs[:, :NC_],
                                        kt_p[:, dk, so * P:(so + 1) * P],
                                        qt_c[:, dk, TC:],
                                        start=(dk == 0), stop=(dk == NS - 1),
                                    )
                                _cp(cs_sb[:, so, :NC_], pcs[:, :NC_])

                        # intra scores^T (both-scaled), triangular-trimmed,
                        # then the constant decay mask
                        at_c = work.tile([P, NS, C], MD, tag="at", name=f"at_{c}")
                        for so in range(NS):
                            off, n = TRIM[so]
                            psc = ppa.tile([P, C], F32, tag="pa",
                                           name=f"pssc_{c}_{so}")
                            for do in range(NS):
                                nc.tensor.matmul(
                                    psc[:, :n],
                                    kt_c[:, do, so * P:(so + 1) * P],
                                    qt_c[:, do, off:off + n],
                                    start=(do == 0), stop=(do == NS - 1),
                                )
                            _tt(at_c[:, so, off:off + n], psc[:, :n],
                                m3_sb[:, so, off:off + n])

                        # retrieved^T = cross + intra (intra trimmed; cross
                        # runs first with start=True over the full tile)
                        rt_c = work.tile([P, NS, C], MD, tag="rt", name=f"rt_{c}")
                        for do in range(NS):
                            pr = ppr.tile([P, C], F32, tag="pr", name=f"psr_{c}_{do}")
                            n_eo = NS if has_state else n_prev
                            for eo in range(n_eo):
                                if has_state:
                                    nc.tensor.matmul(
                                        pr,
                                        S_cur[:, eo, do * P:(do + 1) * P],
                                        qt_c[:, eo, :],
                                        start=(eo == 0), stop=False,
                                    )
                                else:
                                    nc.tensor.matmul(
                                        pr[:, TC:],
                                        v_p[:, eo, do * P:(do + 1) * P],
                                        cs_sb[:, eo, :NC_],
                                        start=(eo == 0), stop=False,
                                    )
                            for so in range(NS):
                                off, n = TRIM[so]
                                nc.tensor.matmul(
                                    pr[:, off:off + n],
                                    v_c[:, so, do * P:(do + 1) * P],
                                    at_c[:, so, off:off + n],
                                    start=False, stop=(so == NS - 1),
                                )
                            _cp(rt_c[:, do, :], pr)

                        # output projection
                        o_sb = work.tile([P, NS, D], F32, tag="o",
                                         bufs=tn["obufs"],
                                         name=f"o_{c}")
                        for ts in range(NS):
                            po = ppa.tile([P, D], F32, tag="pa", name=f"pso_{c}_{ts}")
                            for do in range(NS):
                                nc.tensor.matmul(
                                    po,
                                    rt_c[:, do, ts * P:(ts + 1) * P],
                                    wo_sb[:, do, :],
                                    start=(do == 0), stop=(do == NS - 1),
                                )
                            _cp(o_sb[:, ts, :], po)
                            nc.sync.dma_start(out_t[:, c, ts, :],
                                              o_sb[:, ts, :])

                    if need_kv:
                        kv_prev = (ksc_c, v_c) if has_state else (kt_c, v_c)

            if bench_loop > 1:
                hint = (mybir.EngineType.PE, mybir.EngineType.DVE,
                        mybir.EngineType.Activation, mybir.EngineType.SP,
                        mybir.EngineType.Pool)
                with tc.For_i(0, bench_loop, 1, hint_engines=hint):
                    _chunks()
            else:
                _chunks()

    nc.compile()
    _BUILD_CACHE[key] = nc
    return nc


def _host_prep_v0(x, Wq, Wk, Wv, Wo, decay_logit, out_scale, NE):
    """Shared weights/constants + per-core xT slices."""
    x = np.ascontiguousarray(np.asarray(x, dtype=np.float32))
    gamma = float(1.0 / (1.0 + np.exp(-np.float64(np.asarray(decay_logit)))))
    osc = float(np.asarray(out_scale))

    shared = {
        "wqT": np.ascontiguousarray(np.asarray(Wq, np.float32).T).astype(MD_NP),
        "wkT": np.ascontiguousarray(np.asarray(Wk, np.float32).T).astype(MD_NP),
        "wvT": np.ascontiguousarray(np.asarray(Wv, np.float32).T).astype(MD_NP),
        "woTs": np.ascontiguousarray(
            np.asarray(Wo, np.float32).T * osc).astype(MD_NP),
    }
    j = np.arange(C, dtype=np.float64)
    # ksc[p, so] = gamma^(so*128 + p)
    shared["ksc"] = np.ascontiguousarray(
        (gamma ** j).astype(np.float32).reshape(NS, P).transpose(1, 0))
    shared["qsc"] = np.broadcast_to(
        (gamma ** (C - 1 - j)).astype(np.float32)[None, :], (P, C)).copy()
    jj, ii = np.meshgrid(j, j, indexing="ij")
    m3v = np.where(jj > ii, gamma ** (-C), 0.0).astype(np.float32)
    shared["m3"] = m3v
    shared["ksc2"] = np.broadcast_to(
        (gamma ** j).astype(np.float32)[None, :], (P, C)).copy()
    shared["idn"] = np.eye(P, dtype=np.float32).astype(MD_NP)
    shared["idc"] = (np.eye(P) * (gamma ** C)).astype(np.float32).astype(MD_NP)

    T_ext = NE * C
    in_maps = []
    for core in range(N_CORES):
        b, h = divmod(core, 2)
        start = h * (NL * C)
        xe = np.zeros((T_ext, D), np.float32)
        avail = min(T_ext, T - start)
        xe[:avail] = x[b, start:start + avail]
        m = dict(shared)
        m["xT"] = np.ascontiguousarray(xe.T).astype(MD_NP)
        in_maps.append(m)
    return gamma, in_maps



def _kernel_v0(x, Wq, Wk, Wv, Wo, decay_logit, out_scale):
    gamma = float(1.0 / (1.0 + np.exp(-np.float64(np.asarray(decay_logit)))))
    fast = gamma ** C < 1e-8
    NE, has_state = (NL + 1, False) if fast else (T // C, True)
    nc = _build_v0(NE, has_state, cs_trim=(gamma ** (C // 2) < 1e-4))
    _, in_maps = _host_prep_v0(x, Wq, Wk, Wv, Wo, decay_logit, out_scale, NE)
    res = run_bass_kernel_spmd(
        nc, in_maps, core_ids=list(range(N_CORES)), trace=False)
    result = np.zeros((B, T, D), np.float32)
    for core in range(N_CORES):
        b, h = divmod(core, 2)
        start = h * (NL * C)
        result[b, start:start + NL * C] = res.results[core]["out"]
    return result


def _pick_nd(gamma: float):
    for n in range(1, ND_MAX + 1):
        if gamma ** (128 * n) < 5e-3:
            return n
    return None


def _host_prep_fast(x, Wq, Wk, Wv, Wo, decay_logit, out_scale, ND,
                    algo: str = "h"):
    x = np.ascontiguousarray(np.asarray(x, dtype=np.float32))
    gamma = float(1.0 / (1.0 + np.exp(-np.float64(np.asarray(decay_logit)))))
    osc = float(np.asarray(out_scale))
    SBK = TB + ND
    TLE = SBK * P
    NW = (ND + 1) * P

    G = (np.asarray(Wq, np.float64).T @ np.asarray(Wk, np.float64))
    H = (np.asarray(Wv, np.float64).T @ np.asarray(Wo, np.float64).T) * osc

    s_rel = np.arange(P, dtype=np.int64)[:, None]
    cols = np.arange(NW, dtype=np.int64)[None, :]
    dist = s_rel + (ND - cols // P) * P - (cols % P)
    with np.errstate(over="ignore"):
        mval = np.where(dist >= 1, gamma ** np.maximum(dist - 1, 0), 0.0)
    shared = {
        "Gm": np.ascontiguousarray(G.astype(np.float32)).astype(BF_NP),
        "Hm": np.ascontiguousarray(H.astype(np.float32)).astype(BF_NP),
        "msk": np.ascontiguousarray(mval.astype(np.float32)),
    }

    in_maps = []
    for core in range(N_CORES):
        b, h = divmod(core, 2)
        start = h * TL
        xe = np.zeros((TLE, D), np.float32)
        avail = min(TLE, T - start)
        xe[:avail] = x[b, start:start + avail]
        m = dict(shared)
        # partition-major packed blobs (see _build_fast DRAM layouts):
        # xn[p, sb*D + d] = xe[sb*128 + p, d]
        if algo != "h":
            m["xn"] = np.ascontiguousarray(
                xe.reshape(SBK, P, D).transpose(1, 0, 2).reshape(P, SBK * D)
            ).astype(BF_NP)
        # xT[p, k, eo*(TLE/2) + t'] = xe.T[eo*128 + p, k*(TLE/2) + t']
        xeT = np.ascontiguousarray(xe.T)
        NCH = 2
        TC2 = TLE // NCH
        pm = xeT.reshape(DB, P, TLE).transpose(1, 0, 2)       # [P, DB, TLE]
        m["xT"] = np.ascontiguousarray(
            np.stack([pm[:, :, k * TC2:(k + 1) * TC2].reshape(P, DB * TC2)
                      for k in range(NCH)], axis=1)
        ).astype(BF_NP)
        in_maps.append(m)
    return gamma, in_maps


def kernel(x, Wq, Wk, Wv, Wo, decay_logit, out_scale):
    global LAST_RESULTS
    gamma = float(1.0 / (1.0 + np.exp(-np.float64(np.asarray(decay_logit)))))
    ND = _pick_nd(gamma)
    if ND is None or os.environ.get("KERNEL_PATH") == "v0":
        return _kernel_v0(x, Wq, Wk, Wv, Wo, decay_logit, out_scale)

    algo = os.environ.get("KERNEL_ALGO", "h" if ND == 1 else "rt")
    if ND != 1:
        algo = "rt"
    nc = _build_fast(ND, algo=algo)
    _, in_maps = _host_prep_fast(x, Wq, Wk, Wv, Wo, decay_logit,
                                 out_scale, ND, algo=algo)
    res = run_bass_kernel_spmd(
        nc, in_maps, core_ids=list(range(N_CORES)), trace=False)
    LAST_RESULTS = res

    result = np.zeros((B, T, D), np.float32)
    for core in range(N_CORES):
        b, h = divmod(core, 2)
        blob = np.asarray(res.results[core]["out"], dtype=np.float32)
        result[b, h * TL:(h + 1) * TL] = (
            blob.reshape(P, TB, D).transpose(1, 0, 2).reshape(TL, D))
    return result


# ---------------------------------------------------------------------------
# Benchmarking (dev-only; not used by the grading path).
# ---------------------------------------------------------------------------

def _timed_exec(nc, in_maps, loop: int) -> float:
    """Seconds of wall time for one jitted call with `loop` chained execs."""
    import time

    import jax
    from jax.sharding import Mesh, PartitionSpec
    from jax.experimental.shard_map import shard_map
    from concourse import bass2jax, mybir as _mybir

    n_cores = len(in_maps)
    partition_name = (nc.partition_id_tensor.name
                      if nc.partition_id_tensor else None)
    in_names, out_names, out_avals, zero_outs = [], [], [], []
    for alloc in nc.m.functions[0].allocations:
        if not isinstance(alloc, _mybir.MemoryLocationSet):
            continue
        name = alloc.memorylocations[0].name
        if alloc.kind == "ExternalInput":
            if name != partition_name:
                in_names.append(name)
        elif alloc.kind == "ExternalOutput":
            out_names.append(name)
            shape = tuple(alloc.tensor_shape)
            np_dt = _mybir.dt.np(alloc.dtype)
            out_avals.append(jax.core.ShapedArray(shape, np_dt))
            zero_outs.append(np.zeros(shape, np_dt))

    n_params = len(in_names)
    all_names = in_names + out_names
    if partition_name is not None:
        all_names = all_names + [partition_name]

    def _body(*args):
        ins = list(args[:n_params])
        out_bufs = list(args[n_params:])
        outs = None
        for _ in range(loop):
            operands = ins + out_bufs
            if partition_name is not None:
                operands.append(bass2jax.partition_id_tensor())
            outs = bass2jax._bass_exec_p.bind(
                *operands,
                out_avals=tuple(out_avals),
                in_names=tuple(all_names),
                out_names=tuple(out_names),
                lowering_input_output_aliases=(),
                sim_require_finite=True,
                sim_require_nnan=True,
                nc=nc,
            )
            out_bufs = list(outs)
        return tuple(outs)

    devices = jax.devices()[:n_cores]
    mesh = Mesh(np.asarray(devices), ("core",))
    n_args = n_params + len(out_names)
    sharded = jax.jit(shard_map(
        _body, mesh=mesh,
        in_specs=(PartitionSpec("core"),) * n_args,
        out_specs=(PartitionSpec("core"),) * len(out_names),
        check_rep=False,
    ), keep_unused=True)

    from jax.sharding import NamedSharding
    sh = NamedSharding(mesh, PartitionSpec("core"))
    concat_in = [
        jax.device_put(
            np.concatenate([np.asarray(in_maps[c][name])
                            for c in range(n_cores)], axis=0), sh)
        for name in in_names
    ]
    concat_zero = [
        jax.device_put(
            np.zeros((n_cores * z.shape[0], *z.shape[1:]), z.dtype), sh)
        for z in zero_outs
    ]
    args = concat_in + concat_zero
    jax.block_until_ready(args)
    out = sharded(*args)  # warmup/compile
    jax.block_until_ready(out)
    best = float("inf")
    for _ in range(5):
        t0 = time.perf_counter()
        out = sharded(*args)
        jax.block_until_ready(out)
        best = min(best, time.perf_counter() - t0)
    return best


def bench_exec_ns(x, Wq, Wk, Wv, Wo, decay_logit, out_scale,
                  loops=(1, 101)) -> float:
    gamma = float(1.0 / (1.0 + np.exp(-np.float64(np.asarray(decay_logit)))))
    ND = _pick_nd(gamma)
    assert ND is not None, "bench only supports the fast path"
    algo = os.environ.get("KERNEL_ALGO", "h" if ND == 1 else "rt")
    if ND != 1:
        algo = "rt"
    _, in_maps = _host_prep_fast(x, Wq, Wk, Wv, Wo, decay_logit,
                                 out_scale, ND, algo=algo)
    times = {}
    ncs = {k: _build_fast(ND, bench_loop=k, algo=algo) for k in loops}
    k0, k1 = loops
    # Paired deltas sampled back-to-back, median over pairs: robust against
    # the drifting per-call dispatch floor (tens of ms through the axon
    # tunnel) without cherry-picking favorable fluctuations the way a
    # min-of-deltas would.
    deltas = []
    for _ in range(5):
        t0 = _timed_exec(ncs[k0], in_maps, 1)
        t1 = _timed_exec(ncs[k1], in_maps, 1)
        times[k0] = min(times.get(k0, float("inf")), t0)
        times[k1] = min(times.get(k1, float("inf")), t1)
        if t1 > t0:
            deltas.append(t1 - t0)
    # each hardware iteration of a bench build runs BENCH_BODIES bodies
    per = float(np.median(deltas)) / ((k1 - k0) * BENCH_BODIES)
    return per * 1e9, times



# revision 41
# speedup vs baseline: 1.0891x; 1.0891x over previous
"""Trainium2 Bass kernel for nn_CausalDecayMemory — fast banded path.

Reference (B=4, T=4096, D=512):
    q = x @ Wq.T ; k = x @ Wk.T ; v = x @ Wv.T
    scores[b,t,s] = q[b,t] . k[b,s]
    weights[t,s] = decay^max(s-t-1, 0) for s > t else 0
    out = ((scores * weights) @ v) @ Wo.T * out_scale

Algebraic folding (host-side, free):
    scores = x G x^T       with G = Wq^T Wk
    out    = (A x) H       with H = Wv^T Wo^T * out_scale,  A = scores*weights
This removes two of the four dense projections.

Decay truncation: gamma = sigmoid(decay_logit).  With 128-blocks over t/s,
an s-block only contributes to t-blocks within ND super-diagonals, where
gamma^(128*ND) < tol.  For the graded regime (gamma ~ 0.9526) ND = 1; the
dominant error is the 32-col cross-tile drop (~7e-3 rel) + bf16 (~3.5e-3),
measured 8.2e-3 total vs the 2e-2 gate.

Sharding: pure data-parallel, 8 cores = 4 batches x 2 halves of 2048
positions, each with an ND*128-position lookahead halo of x.  No collectives.

Per-core h-fused pipeline (algo="h", all matmuls bf16, fp32 PSUM):
    g^T[d,t]  = sum_e G[e,d] x^T[e,t]                  (proj 1; N=512)
    h[s,o]    = sum_e x[s,e] H[e,o]  per s-block       (proj 2; N=512,
                depends only on the input DMA -> pure filler work)
    sc[s,t]   = sum_d x^T[d,s] g^T[d,t], banded        (N~224)
    at        = sc * mask                              (DVE, bf16 out)
    out[t,o]  = sum_{j in [tb,tb+ND]} at_j^T h_j       (N=512, 2 MMs/block)
The only x layout needed is x^T (one packed 2.2MB DMA/body); the classic
rt-path (algo="rt") needs x twice (x^T and x-natural, 4.4MB) and an extra
PSUM round-trip rt = at . x, out = rt @ H.

I/O uses partition-major packed DRAM blobs (one contiguous 17KB run per
partition) for full-rate single-descriptor DMA; input x is triple-buffered
in the bench loop so each body's DMA issues ~2.7 bodies ahead of first use.
"""

import os
import sys

import numpy as np

for _p in ("/opt/trn_rl_repo",):
    if _p not in sys.path and os.path.isdir(_p):
        sys.path.insert(0, _p)

import concourse.bass as bass  # noqa: E402
import concourse.mybir as mybir  # noqa: E402
import concourse.tile as tile  # noqa: E402
from concourse import bacc  # noqa: E402
from concourse.bass_utils import run_bass_kernel_spmd  # noqa: E402

B, T, D = 4, 4096, 512
P = 128
TB = 16            # local 128-blocks per core (2048 positions)
TL = TB * P        # 2048
DB = D // P        # 4
N_CORES = 8

F32 = mybir.dt.float32
BF16 = mybir.dt.bfloat16
BF_NP = mybir.dt.np(BF16)

_BUILD_CACHE: dict = {}
LAST_RESULTS = None

ND_MAX = 3         # fast path handles up to 3 super-diagonal blocks


ALL_PARTS = frozenset({"dma_in", "compute", "copies", "dma_out"})
# dev-only: "dma_decoupled" (with "dma_in") redirects input DMAs into
# dummy tiles so compute never depends on them (contention probe).

BENCH_BODIES = 3   # unrolled bodies per bench-loop iteration (== x bufs)
# (6 bodies/iter was tried and is WORSE: 57.4us vs 48.9 — the DMA coupling
# is not per-iteration, and the bigger loop footprint costs throughput.)


def _build_fast(ND: int, bench_loop: int = 1, parts: frozenset = ALL_PARTS,
                unroll: int = 1, algo: str = "h"):
    """algo="h": h-fused pipeline out[t,o] = sum_j at_j . h_j with
    h = x @ H precomputed per s-block (no xn input, no rt stage).
    algo="rt": classic rt = at . x then out = rt @ H."""
    key = ("fast", ND, bench_loop, parts, unroll, algo)
    if key in _BUILD_CACHE:
        return _BUILD_CACHE[key]

    SBK = TB + ND          # s-blocks incl halo
    TLE = SBK * P          # extended positions
    NW = (ND + 1) * P      # mask / at width in t-columns
    USE_H = algo == "h"
    assert not (USE_H and ND != 1), "h-fused schedule is ND=1 only"

    nc = bacc.Bacc("TRN2", target_bir_lowering=False, debug=False)

    # Partition-major packed DRAM layouts: contiguous multi-KB runs per
    # partition -> full-rate DMA (1KB-line rearranges halve throughput).
    # xT is packed as NCH time-staggered chunks so each chunk's refill WAR clears
    # when ITS last reader finishes (spread through the prior body on
    # the same buffer slot) — the write traffic trickles in spread
    # bursts instead of one 2.2MB wall at body start.
    NCH = 2
    TC2 = TLE // NCH
    xT = nc.dram_tensor("xT", [P, NCH, DB * TC2], BF16,
                        kind="ExternalInput").ap()
    xn = (None if USE_H else
          nc.dram_tensor("xn", [P, SBK * D], BF16, kind="ExternalInput").ap())
    Gm = nc.dram_tensor("Gm", [D, D], BF16, kind="ExternalInput").ap()
    Hm = nc.dram_tensor("Hm", [D, D], BF16, kind="ExternalInput").ap()
    msk = nc.dram_tensor("msk", [P, NW], F32, kind="ExternalInput").ap()
    out = nc.dram_tensor("out", [P, TB * D], BF16, kind="ExternalOutput").ap()

    xT_t = xT.rearrange("p k (eo t) -> p k eo t", eo=DB)  # [128, NCH, 4, TC2]
    xn_t = (None if USE_H else
            xn.rearrange("p (sb d) -> p sb d", sb=SBK))  # [128, SBK, 512]
    G_t = Gm.rearrange("(eo p) d -> p eo d", p=P)
    H_t = Hm.rearrange("(eo p) d -> p eo d", p=P)
    out_t = out.rearrange("p (tb d) -> p tb d", tb=TB)  # [128, 16, 512]

    # input x multi-buffer (NBODY-1 body DMA lead)
    NBUF = BENCH_BODIES if (bench_loop > 1 or unroll > 1) else 1

    with tile.TileContext(nc) as tc:
        with (
            tc.tile_pool(name="cpool", bufs=1) as cpool,
            tc.tile_pool(name="dpool", bufs=NBUF) as dpool,
            tc.tile_pool(name="ppa", bufs=6, space="PSUM") as ppa,
            tc.tile_pool(name="ppo", bufs=2, space="PSUM") as ppo,
        ):
            mult = mybir.AluOpType.mult

            G_sb = cpool.tile([P, DB, D], BF16)
            H_sb = cpool.tile([P, DB, D], BF16)
            msk_sb = cpool.tile([P, NW], F32)
            g_sb = cpool.tile([P, DB, TL], BF16)
            at_sb = cpool.tile([P, SBK, NW], BF16)
            if USE_H:
                h_sb = cpool.tile([P, SBK, D], BF16)
                rt_sb = None
            else:
                h_sb = None
                rt_sb = cpool.tile([P, DB, TL], BF16)
            o_sb = cpool.tile([P, TB, D], BF16)

            # t-block coverage of s-block j
            def _cov(j):
                jt0 = max(0, j - ND)
                jt1 = min(TB, j + 1)
                off = (jt0 - j + ND) * P
                return jt0, jt1, off

            DO_DIN = "dma_in" in parts
            DO_MM = "compute" in parts
            DO_CP = "copies" in parts
            DO_DOUT = "dma_out" in parts
            DMA_DECOUPLED = "dma_decoupled" in parts

            # Stripped dev variants: initialize tiles whose producers are
            # disabled (once, outside any bench loop — slope cancels it).
            fixed_x = None
            if DMA_DECOUPLED or not DO_DIN:
                for t in (G_sb, H_sb, msk_sb):
                    nc.vector.memset(t, 0)
                fixed_x = (cpool.tile([P, DB, TLE], BF16, name="fxT"),
                           (None if USE_H else
                            cpool.tile([P, SBK, D], BF16, name="fxn")))
                for t in fixed_x:
                    if t is not None:
                        nc.vector.memset(t, 0)
            if not DO_CP:
                for t in (g_sb, at_sb, rt_sb, h_sb, o_sb):
                    if t is not None:
                        nc.vector.memset(t, 0)

            # The first 32 t-columns of each super-diagonal at-tile hold
            # only weights <= gamma^96 (~8.5e-3): never computed by _sc.
            # The h-fused out-stage reads them as part of full-M lhsT
            # slices, so pin them to zero once here.
            if USE_H and DO_CP:
                nc.vector.memset(at_sb[:, :, 0:32], 0)

            # Loop-invariant constants: one DMA, outside any bench loop.
            if "dma_in" in parts:
                nc.sync.dma_start(G_sb, G_t)
                nc.sync.dma_start(msk_sb, msk)
                nc.sync.dma_start(H_sb, H_t)

            _it = [0]

            def _alloc_x():
                # One rotation slot per call (tag-keyed); with NBUF=3 and
                # three bodies per loop iteration, each slot's refill DMA
                # issues ~2.7 bodies before its consumer.
                it = _it[0]
                _it[0] += 1
                if fixed_x is not None and not DMA_DECOUPLED:
                    return fixed_x
                if DMA_DECOUPLED:
                    return (dpool.tile([P, DB, TLE], BF16, tag="dT",
                                       name=f"dT_sb_{it}"),
                            None if USE_H else
                            dpool.tile([P, SBK, D], BF16, tag="dn",
                                       name=f"dn_sb_{it}"))
                return (dpool.tile([P, DB, TLE], BF16, tag="xT",
                                   name=f"xT_sb_{it}"),
                        None if USE_H else
                        dpool.tile([P, SBK, D], BF16, tag="xn",
                                   name=f"xn_sb_{it}"))

            def _dma_x(tiles):
                # DMA triggers only on the SP and gpsimd/Pool queues — no
                # compute queue ever stalls on a DMA WAR-wait. The NCH
                # chunks (contiguous DRAM source, strided SBUF dest) are
                # spread across BOTH rings: one ring measures only ~180GB/s,
                # so parallel rings halve the per-body transfer time.
                if DO_DIN:
                    dT_sb, dn_sb = tiles
                    for k in range(NCH):
                        eng = nc.sync if k % 2 == 0 else nc.gpsimd
                        eng.dma_start(
                            dT_sb[:, :, k * TC2:(k + 1) * TC2], xT_t[:, k])
                    if dn_sb is not None:
                        nc.gpsimd.dma_start(dn_sb, xn_t)

            def _body(x_tiles):
                if fixed_x is not None:
                    xT_sb, xn_sb = fixed_x
                else:
                    xT_sb, xn_sb = x_tiles

                def _g(c):
                    cr = slice(c * 512, (c + 1) * 512)
                    for do in range(DB):
                        pg = ppa.tile([P, 512], F32, tag="pa",
                                      name=f"pg_{c}_{do}")
                        if DO_MM:
                            for e in range(DB):
                                nc.tensor.matmul(
                                    pg, G_sb[:, e, do * P:(do + 1) * P],
                                    xT_sb[:, e, cr],
                                    start=(e == 0), stop=(e == DB - 1))
                        if DO_CP:
                            nc.vector.tensor_copy(out=g_sb[:, do, cr], in_=pg)

                def _sc(j):
                    jt0, jt1, off = _cov(j)
                    n = (jt1 - jt0) * P
                    # skip the first 32 t-cols of super-diagonal tiles
                    # (weight <= gamma^96; at_sb pinned to zero there)
                    lo = off + 32 if off == 0 else off
                    ps = ppa.tile([P, 512], F32, tag="pa", name=f"ps_{j}")
                    if DO_MM:
                        for d in range(DB):
                            nc.tensor.matmul(
                                ps[:, lo:off + n],
                                xT_sb[:, d, j * P:(j + 1) * P],
                                g_sb[:, d, jt0 * P + lo - off:jt1 * P],
                                start=(d == 0), stop=(d == DB - 1))
                    if DO_CP:
                        nc.vector.tensor_tensor(
                            out=at_sb[:, j, lo:off + n],
                            in0=ps[:, lo:off + n],
                            in1=msk_sb[:, lo:off + n], op=mult)

                def _rt(blist):
                    if not blist:
                        return
                    nb = len(blist)
                    for do in range(DB):
                        pr = ppa.tile([P, 512], F32, tag="pa",
                                      name=f"pr_{blist[0]}_{do}")
                        if DO_MM:
                            for ii, i in enumerate(blist):
                                for j in range(i, i + ND + 1):
                                    # skip the zero region: the first 32
                                    # t-cols of super-diagonal at-tiles are
                                    # never computed by _sc
                                    lo = 32 if (j == i + ND and ND > 0) else 0
                                    nc.tensor.matmul(
                                        pr[:, ii * P + lo:(ii + 1) * P],
                                        xn_sb[:, j, do * P:(do + 1) * P],
                                        at_sb[:, j,
                                              (i - j + ND) * P + lo:
                                              (i - j + ND + 1) * P],
                                        start=(j == i), stop=(j == i + ND))
                        if DO_CP:
                            nc.scalar.copy(
                                out=rt_sb[:, do,
                                          blist[0] * P:blist[0] * P + nb * P],
                                in_=pr[:, :nb * P])

                def _out(blist):
                    for tb in blist:
                        po = ppo.tile([P, D], F32, tag="po", name=f"po_{tb}")
                        if DO_MM:
                            for do in range(DB):
                                nc.tensor.matmul(
                                    po, rt_sb[:, do, tb * P:(tb + 1) * P],
                                    H_sb[:, do, :],
                                    start=(do == 0), stop=(do == DB - 1))
                        if DO_CP:
                            nc.scalar.copy(out=o_sb[:, tb, :], in_=po)
                    if DO_DOUT and blist:
                        b0, b1 = blist[0], blist[-1] + 1
                        nc.gpsimd.dma_start(out_t[:, b0:b1, :],
                                            o_sb[:, b0:b1, :])

                def _h(j):
                    # h[s, o] = sum_e x[s, e] H[e, o] for s-block j
                    ph = ppa.tile([P, D], F32, tag="pa", name=f"ph_{j}")
                    if DO_MM:
                        for e in range(DB):
                            nc.tensor.matmul(
                                ph, xT_sb[:, e, j * P:(j + 1) * P],
                                H_sb[:, e, :],
                                start=(e == 0), stop=(e == DB - 1))
                    if DO_CP:
                        nc.scalar.copy(out=h_sb[:, j, :], in_=ph)

                def _out2(blist):
                    # out[t, o] = sum_{j in [tb, tb+ND]} at_j[:, tcols].T @ h_j
                    # (at's zero-pinned cols contribute 0 to the first rows)
                    for tb in blist:
                        po = ppo.tile([P, D], F32, tag="po", name=f"po_{tb}")
                        if DO_MM:
                            for j in range(tb, tb + ND + 1):
                                nc.tensor.matmul(
                                    po,
                                    at_sb[:, j, (tb - j + ND) * P:
                                          (tb - j + ND + 1) * P],
                                    h_sb[:, j, :],
                                    start=(j == tb), stop=(j == tb + ND))
                        if DO_CP:
                            nc.scalar.copy(out=o_sb[:, tb, :], in_=po)
                    if DO_DOUT and blist:
                        b0, b1 = blist[0], blist[-1] + 1
                        nc.gpsimd.dma_start(out_t[:, b0:b1, :],
                                            o_sb[:, b0:b1, :])

                if USE_H:
                    # h(j) depends only on the input DMA -> pure filler work.
                    # sc emissions are interleaved with h so the DVE (which
                    # drains one at-mult per ~660ns) is never asked for more
                    # than one per ~1.2us of PE work, and every _out2 batch
                    # has >=1.5us of PE slack after the at/h tiles it reads.
                    _g(0)
                    _g(1)
                    _h(0)
                    _sc(0)
                    _h(1)
                    _sc(1)
                    _h(2)
                    _sc(2)
                    _h(3)
                    _sc(3)
                    _h(4)
                    _g(2)
                    _sc(4)
                    _h(5)
                    _sc(5)
                    _h(6)
                    _sc(6)
                    _out2([0, 1])
                    _sc(7)
                    _out2([2, 3])
                    _g(3)
                    _sc(8)
                    _h(7)
                    _sc(9)
                    _h(8)
                    _sc(10)
                    _h(9)
                    _out2([4, 5])
                    _sc(11)
                    _h(10)
                    _out2([6, 7])
                    _sc(12)
                    _h(11)
                    _sc(13)
                    _h(12)
                    _sc(14)
                    _h(13)
                    _out2([8, 9])
                    _sc(15)
                    _h(14)
                    for j in range(TB, TB + ND):
                        _sc(j)
                    _h(15)
                    _h(16)
                    _out2([10, 11, 12, 13])
                    _out2([14, 15])
                else:
                    # rt batches are shifted by ND blocks so batch k only
                    # needs at-tiles <= 4k+3 (emitted just before).
                    rtg = [list(range(max(0, 4 * k - ND), 4 * (k + 1) - ND))
                           for k in range(4)] + [list(range(16 - ND, 16))]
                    _g(0)
                    _g(1)
                    for j in range(0, 4):
                        _sc(j)
                    for c in (1, 2, 3):
                        if c < 3:
                            _g(c + 1)
                        _rt(rtg[c - 1])
                        for j in range(4 * c, 4 * c + 4):
                            _sc(j)
                        if c == 3:
                            for j in range(TB, TB + ND):
                                _sc(j)
                        _out(rtg[c - 1])
                    _rt(rtg[3])
                    _rt(rtg[4])
                    _out(rtg[3])
                    _out(rtg[4])

            if bench_loop > 1:
                # BENCH_BODIES unrolled bodies per hardware iteration with
                # as many x-buffers: each body's input DMA is issued
                # ~NBODY-1 bodies before its first consumer, so transfers
                # overlap compute, and any conservative per-iteration loop
                # sync is amortized over NBODY bodies.
                # Effective executions per run: BENCH_BODIES * bench_loop.
                hint = (mybir.EngineType.PE, mybir.EngineType.DVE,
                        mybir.EngineType.Activation, mybir.EngineType.SP,
                        mybir.EngineType.Pool)
                ring = []
                for _ in range(BENCH_BODIES - 1):
                    t = _alloc_x()
                    _dma_x(t)
                    ring.append(t)
                with tc.For_i(0, bench_loop, 1, hint_engines=hint):
                    for i in range(BENCH_BODIES):
                        t = _alloc_x()
                        _dma_x(t)
                        ring.append(t)
                        _body(ring[i])
            else:
                for _ in range(unroll):
                    a = _alloc_x()
                    _dma_x(a)
                    _body(a)

    nc.compile()
    _BUILD_CACHE[key] = nc
    return nc


# ---------------------------------------------------------------------------
# Exact fallback path (v0): RetNet-style chunked-decay recurrence with
# carried KV state.  Used only when gamma is too close to 1 for the
# banded fast path (ND > ND_MAX).  Verbatim from the previous kernel.
# ---------------------------------------------------------------------------
C = 512          # super-chunk length
NS = 4           # 128-sub-tiles per 512
NL = 4           # local super-chunks per core (2048 positions)

# Matmul input dtype: float32r streams 4x faster than float32 on the PE at
# N>=256 (single-pass relaxed-precision fp32); same bit layout as fp32.
# KERNEL_DT: "f32r" (default) | "f32" | "bf16"
_DT_MODE = os.environ.get("KERNEL_DT",
                          "f32" if os.environ.get("KERNEL_F32") == "1"
                          else "f32r")
USE_F32R = _DT_MODE == "f32r"



MD = {"f32r": mybir.dt.float32r, "f32": F32,
      "bf16": mybir.dt.bfloat16}[_DT_MODE]  # matmul-input dtype
MD_NP = mybir.dt.np(MD)


TUNE = {
    "ppa": 4, "ppr": 4, "kt": "mm", "eng": "vec", "odma": "sync", "obufs": 1,
    "work": 2, "proj": 2, "state": 2,
}


def _build_v0(NE: int, has_state: bool, bench_loop: int = 1, tune: dict | None = None,
           cs_trim: bool = True):
    """Build + compile the per-core Bass program. NE = total super-chunks
    (NL local + lookahead tail); has_state = carry decayed KV state across
    chunks (exact for any gamma) vs. single-chunk truncation. bench_loop > 1
    wraps the body in an on-device loop (timing use only)."""
    tn = dict(TUNE)
    if tune:
        tn.update(tune)
    key = (NE, has_state, _DT_MODE, bench_loop, cs_trim, tuple(sorted(tn.items())))
    if key in _BUILD_CACHE:
        return _BUILD_CACHE[key]

    nc = bacc.Bacc("TRN2", target_bir_lowering=False, debug=False)

    xT = nc.dram_tensor("xT", [D, NE * C], MD, kind="ExternalInput").ap()
    wqT = nc.dram_tensor("wqT", [D, D], MD, kind="ExternalInput").ap()
    wkT = nc.dram_tensor("wkT", [D, D], MD, kind="ExternalInput").ap()
    wvT = nc.dram_tensor("wvT", [D, D], MD, kind="ExternalInput").ap()
    woTs = nc.dram_tensor("woTs", [D, D], MD, kind="ExternalInput").ap()
    m3 = nc.dram_tensor("m3", [C, C], F32, kind="ExternalInput").ap()
    qsc = nc.dram_tensor("qsc", [P, C], F32, kind="ExternalInput").ap()
    ksc = nc.dram_tensor("ksc", [P, NS], F32, kind="ExternalInput").ap()
    ksc2 = nc.dram_tensor("ksc2", [P, C], F32, kind="ExternalInput").ap()
    idn = nc.dram_tensor("idn", [P, P], MD, kind="ExternalInput").ap()
    idc = nc.dram_tensor("idc", [P, P], MD, kind="ExternalInput").ap()
    out = nc.dram_tensor("out", [NL * C, D], F32, kind="ExternalOutput").ap()

    xT_t = xT.rearrange("(eo p) t -> p eo t", p=P)          # [128, 4, NE*C]
    wq_t = wqT.rearrange("(eo p) d -> p eo d", p=P)
    wk_t = wkT.rearrange("(eo p) d -> p eo d", p=P)
    wv_t = wvT.rearrange("(eo p) d -> p eo d", p=P)
    wo_t = woTs.rearrange("(eo p) d -> p eo d", p=P)
    m3_t = m3.rearrange("(so p) t -> p so t", p=P)
    out_t = out.rearrange("(c ts p) d -> p c ts d", p=P, ts=NS)

    with tile.TileContext(nc) as tc:
        with (
            tc.tile_pool(name="wpool", bufs=1) as wpool,
            tc.tile_pool(name="cpool", bufs=1) as cpool,
            tc.tile_pool(name="state", bufs=tn["state"]) as state,
            tc.tile_pool(name="proj", bufs=tn["proj"]) as proj,
            tc.tile_pool(name="work", bufs=tn["work"]) as work,
            tc.tile_pool(name="ppa", bufs=tn["ppa"], space="PSUM") as ppa,
            tc.tile_pool(name="ppr", bufs=tn["ppr"], space="PSUM") as ppr,
        ):
            mult = mybir.AluOpType.mult
            _rr = [0]

            def _eng():
                if tn["eng"] == "any":
                    return nc.any
                if tn["eng"] == "vec":
                    return nc.vector
                _rr[0] ^= 1
                return nc.vector if _rr[0] else nc.scalar

            def _cp(out, in_):
                e = _eng()
                if e is nc.scalar:
                    nc.scalar.copy(out=out, in_=in_)
                else:
                    e.tensor_copy(out=out, in_=in_)

            def _tt(out, in0, in1):
                e = _eng()
                if e is nc.scalar:
                    e = nc.vector   # ACT has no general tensor_tensor
                e.tensor_tensor(out=out, in0=in0, in1=in1, op=mult)

            wq_sb = wpool.tile([P, NS, D], MD)
            nc.sync.dma_start(wq_sb, wq_t)
            wk_sb = wpool.tile([P, NS, D], MD)
            nc.sync.dma_start(wk_sb, wk_t)
            wv_sb = wpool.tile([P, NS, D], MD)
            nc.sync.dma_start(wv_sb, wv_t)
            wo_sb = wpool.tile([P, NS, D], MD)
            nc.sync.dma_start(wo_sb, wo_t)
            m3_sb = cpool.tile([P, NS, C], F32)
            nc.sync.dma_start(m3_sb, m3_t)
            qsc_sb = cpool.tile([P, C], F32)
            nc.sync.dma_start(qsc_sb, qsc)
            ksc_sb = cpool.tile([P, NS], F32)
            nc.sync.dma_start(ksc_sb, ksc)
            ksc2_sb = cpool.tile([P, C], F32)
            nc.sync.dma_start(ksc2_sb, ksc2)
            idn_sb = cpool.tile([P, P], MD)
            nc.sync.dma_start(idn_sb, idn)
            idc_sb = cpool.tile([P, P], MD)
            nc.sync.dma_start(idc_sb, idc)

            def _chunks():
                kv_prev = None   # (kT, v) [fast] or (kscaled, v) [general]
                S_prev = None    # state tile (general path only)
                # triangular trim: scores/intra block so only needs
                # t in (so*128 - 256, (so+1)*128) -- the decay window bound
                # applies below as well when cs_trim; keep N >= 256 for
                # full-rate fp32r
                if cs_trim and not has_state:
                    TRIM = [(0, 256), (0, 256), (0, 384), (P, 384)]
                else:
                    TRIM = [(0, max(256, (so + 1) * P)) for so in range(NS)]
                for c in range(NE - 1, -1, -1):
                    local = c < NL
                    need_kv = c > 0 or local

                    halo_trim = (not has_state) and cs_trim and c == NE - 1
                    nh = C // 2 if halo_trim else C
                    xt = work.tile([P, NS, C], MD, tag="xt", name=f"xt_{c}")
                    nc.sync.dma_start(xt[:, :, :nh],
                                      xT_t[:, :, c * C:c * C + nh])

                    # ---- general path: scaled-natural k + decayed state S ----
                    if has_state and kv_prev is not None:
                        ksc_p, v_p = kv_prev
                        S_cur = state.tile([P, NS, D], MD, tag="S", name=f"S_{c}")
                        for eo in range(NS):
                            ps = ppa.tile([P, D], F32, tag="pa", name=f"psS_{c}_{eo}")
                            with_id = S_prev is not None
                            for so in range(NS):
                                nc.tensor.matmul(
                                    ps,
                                    ksc_p[:, so, eo * P:(eo + 1) * P],
                                    v_p[:, so, :],
                                    start=(so == 0),
                                    stop=(so == NS - 1 and not with_id),
                                )
                            if with_id:
                                nc.tensor.matmul(
                                    ps, idc_sb, S_prev[:, eo, :],
                                    start=False, stop=True,
                                )
                            _cp(S_cur[:, eo, :], ps)
                        S_prev = S_cur

                    if has_state and need_kv:
                        ksc_c = proj.tile([P, NS, D], MD, tag="ksc", name=f"ksc_{c}")
                        for so in range(NS):
                            pk = ppa.tile([P, D], F32, tag="pa", name=f"psk_{c}_{so}")
                            for eo in range(NS):
                                nc.tensor.matmul(
                                    pk,
                                    xt[:, eo, so * P:(so + 1) * P],
                                    wk_sb[:, eo, :],
                                    start=(eo == 0), stop=(eo == NS - 1),
                                )
                            _tt(ksc_c[:, so, :], pk,
                                ksc_sb[:, so:so + 1].to_broadcast((P, D)))

                    # ---- shared: v natural; scaled k^T (fast: all chunks) ----
                    if need_kv:
                        n_vso = (NS // 2 if ((not has_state) and cs_trim
                                             and c == NE - 1) else NS)
                        v_c = proj.tile([P, NS, D], MD, tag="v", name=f"v_{c}")
                        for so in range(n_vso):
                            pv = ppa.tile([P, D], F32, tag="pa", name=f"psv_{c}_{so}")
                            for eo in range(NS):
                                nc.tensor.matmul(
                                    pv,
                                    xt[:, eo, so * P:(so + 1) * P],
                                    wv_sb[:, eo, :],
                                    start=(eo == 0), stop=(eo == NS - 1),
                                )
                            _cp(v_c[:, so, :], pv)

                    # halo chunk only feeds the cross path, whose weight
                    # is < gamma^256 beyond its first 256 positions
                    if need_kv and (local or not has_state):
                        kt_c = work.tile([P, NS, C], MD, tag="kt", name=f"kt_{c}")
                        for do in range(NS):
                            pk2 = ppa.tile([P, C], F32, tag="pa",
                                           name=f"pskt_{c}_{do}")
                            for ei in range(NS):
                                nc.tensor.matmul(
                                    pk2[:, :nh],
                                    wk_sb[:, ei, do * P:(do + 1) * P],
                                    xt[:, ei, :nh],
                                    start=(ei == 0), stop=(ei == NS - 1),
                                )
                            _tt(kt_c[:, do, :nh], pk2[:, :nh], ksc2_sb[:, :nh])

                    if local:
                        # scaled q^T: qt[e, t] with gamma^(C-1-i) folded in
                        qt_c = work.tile([P, NS, C], MD, tag="qt", name=f"qt_{c}")
                        for eo in range(NS):
                            pq = ppa.tile([P, C], F32, tag="pa", name=f"psq_{c}_{eo}")
                            for ei in range(NS):
                                nc.tensor.matmul(
                                    pq,
                                    wq_sb[:, ei, eo * P:(eo + 1) * P],
                                    xt[:, ei, :],
                                    start=(ei == 0), stop=(ei == NS - 1),
                                )
                            _tt(qt_c[:, eo, :], pq, qsc_sb)

                        # fast path: cross-chunk scores cs[s', t] =
                        # (K~_prev Q~_c) using the transposed k of chunk c+1;
                        # cross then becomes V_prev^T @ cs (no natural k, no S)
                        if not has_state:
                            # cross weight <= gamma^(C - TC) for t < TC, so
                            # the t < TC half can be dropped when gamma is
                            # small enough (cs_trim)
                            TC = C // 2 if cs_trim else 0
                            NC_ = C - TC
                            kt_p, v_p = kv_prev
                            n_prev = (NS // 2 if (cs_trim and c == NL - 1
                                                  and NE == NL + 1) else NS)
                            cs_sb = state.tile([P, NS, C], MD, tag="S",
                                               name=f"cs_{c}")
                            for so in range(n_prev):
                                pcs = ppa.tile([P, C], F32, tag="pa",
                                               name=f"pscs_{c}_{so}")
                                for dk in range(NS):
                                    nc.tensor.matmul(
                                        pcs[:, :NC_],
                                        kt_p[:, dk, so * P:(so + 1) * P],
                                        qt_c[:, dk, TC:],
                                        start=(dk == 0), stop=(dk == NS - 1),
                                    )
                                _cp(cs_sb[:, so, :NC_], pcs[:, :NC_])

                        # intra scores^T (both-scaled), triangular-trimmed,
                        # then the constant decay mask
                        at_c = work.tile([P, NS, C], MD, tag="at", name=f"at_{c}")
                        for so in range(NS):
                            off, n = TRIM[so]
                            psc = ppa.tile([P, C], F32, tag="pa",
                                           name=f"pssc_{c}_{so}")
                            for do in range(NS):
                                nc.tensor.matmul(
                                    psc[:, :n],
                                    kt_c[:, do, so * P:(so + 1) * P],
                                    qt_c[:, do, off:off + n],
                                    start=(do == 0), stop=(do == NS - 1),
                                )
                            _tt(at_c[:, so, off:off + n], psc[:, :n],
                                m3_sb[:, so, off:off + n])

                        # retrieved^T = cross + intra (intra trimmed; cross
                        # runs first with start=True over the full tile)
                        rt_c = work.tile([P, NS, C], MD, tag="rt", name=f"rt_{c}")
                        for do in range(NS):
                            pr = ppr.tile([P, C], F32, tag="pr", name=f"psr_{c}_{do}")
                            n_eo = NS if has_state else n_prev
                            for eo in range(n_eo):
                                if has_state:
                                    nc.tensor.matmul(
                                        pr,
                                        S_cur[:, eo, do * P:(do + 1) * P],
                                        qt_c[:, eo, :],
                                        start=(eo == 0), stop=False,
                                    )
                                else:
                                    nc.tensor.matmul(
                                        pr[:, TC:],
                                        v_p[:, eo, do * P:(do + 1) * P],
                                        cs_sb[:, eo, :NC_],
                                        start=(eo == 0), stop=False,
                                    )
                            for so in range(NS):
                                off, n = TRIM[so]
                                nc.tensor.matmul(
                                    pr[:, off:off + n],
                                    v_c[:, so, do * P:(do + 1) * P],
                                    at_c[:, so, off:off + n],
                                    start=False, stop=(so == NS - 1),
                                )
                            _cp(rt_c[:, do, :], pr)

                        # output projection
                        o_sb = work.tile([P, NS, D], F32, tag="o",
                                         bufs=tn["obufs"],
                                         name=f"o_{c}")
                        for ts in range(NS):
                            po = ppa.tile([P, D], F32, tag="pa", name=f"pso_{c}_{ts}")
                            for do in range(NS):
                                nc.tensor.matmul(
                                    po,
                                    rt_c[:, do, ts * P:(ts + 1) * P],
                                    wo_sb[:, do, :],
                                    start=(do == 0), stop=(do == NS - 1),
                                )
                            _cp(o_sb[:, ts, :], po)
                            nc.sync.dma_start(out_t[:, c, ts, :],
                                              o_sb[:, ts, :])

                    if need_kv:
                        kv_prev = (ksc_c, v_c) if has_state else (kt_c, v_c)

            if bench_loop > 1:
                hint = (mybir.EngineType.PE, mybir.EngineType.DVE,
                        mybir.EngineType.Activation, mybir.EngineType.SP,
                        mybir.EngineType.Pool)
                with tc.For_i(0, bench_loop, 1, hint_engines=hint):
                    _chunks()
            else:
                _chunks()

    nc.compile()
    _BUILD_CACHE[key] = nc
    return nc


def _host_prep_v0(x, Wq, Wk, Wv, Wo, decay_logit, out_scale, NE):
    """Shared weights/constants + per-core xT slices."""
    x = np.ascontiguousarray(np.asarray(x, dtype=np.float32))
    gamma = float(1.0 / (1.0 + np.exp(-np.float64(np.asarray(decay_logit)))))
    osc = float(np.asarray(out_scale))

    shared = {
        "wqT": np.ascontiguousarray(np.asarray(Wq, np.float32).T).astype(MD_NP),
        "wkT": np.ascontiguousarray(np.asarray(Wk, np.float32).T).astype(MD_NP),
        "wvT": np.ascontiguousarray(np.asarray(Wv, np.float32).T).astype(MD_NP),
        "woTs": np.ascontiguousarray(
            np.asarray(Wo, np.float32).T * osc).astype(MD_NP),
    }
    j = np.arange(C, dtype=np.float64)
    # ksc[p, so] = gamma^(so*128 + p)
    shared["ksc"] = np.ascontiguousarray(
        (gamma ** j).astype(np.float32).reshape(NS, P).transpose(1, 0))
    shared["qsc"] = np.broadcast_to(
        (gamma ** (C - 1 - j)).astype(np.float32)[None, :], (P, C)).copy()
    jj, ii = np.meshgrid(j, j, indexing="ij")
    m3v = np.where(jj > ii, gamma ** (-C), 0.0).astype(np.float32)
    shared["m3"] = m3v
    shared["ksc2"] = np.broadcast_to(
        (gamma ** j).astype(np.float32)[None, :], (P, C)).copy()
    shared["idn"] = np.eye(P, dtype=np.float32).astype(MD_NP)
    shared["idc"] = (np.eye(P) * (gamma ** C)).astype(np.float32).astype(MD_NP)

    T_ext = NE * C
    in_maps = []
    for core in range(N_CORES):
        b, h = divmod(core, 2)
        start = h * (NL * C)
        xe = np.zeros((T_ext, D), np.float32)
        avail = min(T_ext, T - start)
        xe[:avail] = x[b, start:start + avail]
        m = dict(shared)
        m["xT"] = np.ascontiguousarray(xe.T).astype(MD_NP)
        in_maps.append(m)
    return gamma, in_maps



def _kernel_v0(x, Wq, Wk, Wv, Wo, decay_logit, out_scale):
    gamma = float(1.0 / (1.0 + np.exp(-np.float64(np.asarray(decay_logit)))))
    fast = gamma ** C < 1e-8
    NE, has_state = (NL + 1, False) if fast else (T // C, True)
    nc = _build_v0(NE, has_state, cs_trim=(gamma ** (C // 2) < 1e-4))
    _, in_maps = _host_prep_v0(x, Wq, Wk, Wv, Wo, decay_logit, out_scale, NE)
    res = run_bass_kernel_spmd(
        nc, in_maps, core_ids=list(range(N_CORES)), trace=False)
    result = np.zeros((B, T, D), np.float32)
    for core in range(N_CORES):
        b, h = divmod(core, 2)
        start = h * (NL * C)
        result[b, start:start + NL * C] = res.results[core]["out"]
    return result


def _pick_nd(gamma: float):
    for n in range(1, ND_MAX + 1):
        if gamma ** (128 * n) < 5e-3:
            return n
    return None


def _host_prep_fast(x, Wq, Wk, Wv, Wo, decay_logit, out_scale, ND,
                    algo: str = "h"):
    x = np.ascontiguousarray(np.asarray(x, dtype=np.float32))
    gamma = float(1.0 / (1.0 + np.exp(-np.float64(np.asarray(decay_logit)))))
    osc = float(np.asarray(out_scale))
    SBK = TB + ND
    TLE = SBK * P
    NW = (ND + 1) * P

    G = (np.asarray(Wq, np.float64).T @ np.asarray(Wk, np.float64))
    H = (np.asarray(Wv, np.float64).T @ np.asarray(Wo, np.float64).T) * osc

    s_rel = np.arange(P, dtype=np.int64)[:, None]
    cols = np.arange(NW, dtype=np.int64)[None, :]
    dist = s_rel + (ND - cols // P) * P - (cols % P)
    with np.errstate(over="ignore"):
        mval = np.where(dist >= 1, gamma ** np.maximum(dist - 1, 0), 0.0)
    shared = {
        "Gm": np.ascontiguousarray(G.astype(np.float32)).astype(BF_NP),
        "Hm": np.ascontiguousarray(H.astype(np.float32)).astype(BF_NP),
        "msk": np.ascontiguousarray(mval.astype(np.float32)),
    }

    in_maps = []
    for core in range(N_CORES):
        b, h = divmod(core, 2)
        start = h * TL
        xe = np.zeros((TLE, D), np.float32)
        avail = min(TLE, T - start)
        xe[:avail] = x[b, start:start + avail]
        m = dict(shared)
        # partition-major packed blobs (see _build_fast DRAM layouts):
        # xn[p, sb*D + d] = xe[sb*128 + p, d]
        if algo != "h":
            m["xn"] = np.ascontiguousarray(
                xe.reshape(SBK, P, D).transpose(1, 0, 2).reshape(P, SBK * D)
            ).astype(BF_NP)
        # xT[p, k, eo*(TLE/2) + t'] = xe.T[eo*128 + p, k*(TLE/2) + t']
        xeT = np.ascontiguousarray(xe.T)
        NCH = 2
        TC2 = TLE // NCH
        pm = xeT.reshape(DB, P, TLE).transpose(1, 0, 2)       # [P, DB, TLE]
        m["xT"] = np.ascontiguousarray(
            np.stack([pm[:, :, k * TC2:(k + 1) * TC2].reshape(P, DB * TC2)
                      for k in range(NCH)], axis=1)
        ).astype(BF_NP)
        in_maps.append(m)
    return gamma, in_maps


def kernel(x, Wq, Wk, Wv, Wo, decay_logit, out_scale):
    global LAST_RESULTS
    gamma = float(1.0 / (1.0 + np.exp(-np.float64(np.asarray(decay_logit)))))
    ND = _pick_nd(gamma)
    if ND is None or os.environ.get("KERNEL_PATH") == "v0":
        return _kernel_v0(x, Wq, Wk, Wv, Wo, decay_logit, out_scale)

    algo = os.environ.get("KERNEL_ALGO", "h" if ND == 1 else "rt")
    if ND != 1:
        algo = "rt"
    nc = _build_fast(ND, algo=algo)
    _, in_maps = _host_prep_fast(x, Wq, Wk, Wv, Wo, decay_logit,
                                 out_scale, ND, algo=algo)
    res = run_bass_kernel_spmd(
        nc, in_maps, core_ids=list(range(N_CORES)), trace=False)
    LAST_RESULTS = res

    result = np.zeros((B, T, D), np.float32)
    for core in range(N_CORES):
        b, h = divmod(core, 2)
        blob = np.asarray(res.results[core]["out"], dtype=np.float32)
        result[b, h * TL:(h + 1) * TL] = (
            blob.reshape(P, TB, D).transpose(1, 0, 2).reshape(TL, D))
    return result


# ---------------------------------------------------------------------------
# Benchmarking (dev-only; not used by the grading path).
# ---------------------------------------------------------------------------

def _timed_exec(nc, in_maps, loop: int) -> float:
    """Seconds of wall time for one jitted call with `loop` chained execs."""
    import time

    import jax
    from jax.sharding import Mesh, PartitionSpec
    from jax.experimental.shard_map import shard_map
    from concourse import bass2jax, mybir as _mybir

    n_cores = len(in_maps)
    partition_name = (nc.partition_id_tensor.name
                      if nc.partition_id_tensor else None)
    in_names, out_names, out_avals, zero_outs = [], [], [], []
    for alloc in nc.m.functions[0].allocations:
        if not isinstance(alloc, _mybir.MemoryLocationSet):
            continue
        name = alloc.memorylocations[0].name
        if alloc.kind == "ExternalInput":
            if name != partition_name:
                in_names.append(name)
        elif alloc.kind == "ExternalOutput":
            out_names.append(name)
            shape = tuple(alloc.tensor_shape)
            np_dt = _mybir.dt.np(alloc.dtype)
            out_avals.append(jax.core.ShapedArray(shape, np_dt))
            zero_outs.append(np.zeros(shape, np_dt))

    n_params = len(in_names)
    all_names = in_names + out_names
    if partition_name is not None:
        all_names = all_names + [partition_name]

    def _body(*args):
        ins = list(args[:n_params])
        out_bufs = list(args[n_params:])
        outs = None
        for _ in range(loop):
            operands = ins + out_bufs
            if partition_name is not None:
                operands.append(bass2jax.partition_id_tensor())
            outs = bass2jax._bass_exec_p.bind(
                *operands,
                out_avals=tuple(out_avals),
                in_names=tuple(all_names),
                out_names=tuple(out_names),
                lowering_input_output_aliases=(),
                sim_require_finite=True,
                sim_require_nnan=True,
                nc=nc,
            )
            out_bufs = list(outs)
        return tuple(outs)

    devices = jax.devices()[:n_cores]
    mesh = Mesh(np.asarray(devices), ("core",))
    n_args = n_params + len(out_names)
    sharded = jax.jit(shard_map(
        _body, mesh=mesh,
        in_specs=(PartitionSpec("core"),) * n_args,
        out_specs=(PartitionSpec("core"),) * len(out_names),
        check_rep=False,
    ), keep_unused=True)

    from jax.sharding import NamedSharding
    sh = NamedSharding(mesh, PartitionSpec("core"))
    concat_in = [
        jax.device_put(
            np.concatenate([np.asarray(in_maps[c][name])
                            for c in range(n_cores)], axis=0), sh)
        for name in in_names
    ]
    concat_zero = [
        jax.device_put(
            np.zeros((n_cores * z.shape[0], *z.shape[1:]), z.dtype), sh)
        for z in zero_outs
    ]
    args = concat_in + concat_zero
    jax.block_until_ready(args)
    out = sharded(*args)  # warmup/compile
    jax.block_until_ready(out)
    best = float("inf")
    for _ in range(5):
        t0 = time.perf_counter()
        out = sharded(*args)
        jax.block_until_ready(out)
        best = min(best, time.perf_counter() - t0)
    return best


def bench_exec_ns(x, Wq, Wk, Wv, Wo, decay_logit, out_scale,
                  loops=(1, 101)) -> float:
    gamma = float(1.0 / (1.0 + np.exp(-np.float64(np.asarray(decay_logit)))))
    ND = _pick_nd(gamma)
    assert ND is not None, "bench only supports the fast path"
    algo = os.environ.get("KERNEL_ALGO", "h" if ND == 1 else "rt")
    if ND != 1:
        algo = "rt"
    _, in_maps = _host_prep_fast(x, Wq, Wk, Wv, Wo, decay_logit,
                                 out_scale, ND, algo=algo)
    times = {}
    ncs = {k: _build_fast(ND, bench_loop=k, algo=algo) for k in loops}
    k0, k1 = loops
    # Paired deltas sampled back-to-back, median over pairs: robust against
    # the drifting per-call dispatch floor (tens of ms through the axon
    # tunnel) without cherry-picking favorable fluctuations the way a
    # min-of-deltas would.
    deltas = []
    for _ in range(5):
        t0 = _timed_exec(ncs[k0], in_maps, 1)
        t1 = _timed_exec(ncs[k1], in_maps, 1)
        times[k0] = min(times.get(k0, float("inf")), t0)
        times[k1] = min(times.get(k1, float("inf")), t1)
        if t1 > t0:
            deltas.append(t1 - t0)
    # each hardware iteration of a bench build runs BENCH_BODIES bodies
    per = float(np.median(deltas)) / ((k1 - k0) * BENCH_BODIES)
    return per * 1e9, times

